# revision 1
# baseline (speedup 1.0000x reference)
"""DBOT Sinkhorn loss kernel for 8 TRN2 NeuronCores.

Strategy: P0 = exp(s*img@txt.T - 1) is row-sharded (1024 rows/core) and kept
SBUF-resident in bf16 (16MB/core). Sinkhorn iterations factor exactly as
P = diag(r) P0 diag(c), so each iteration is two matvec passes:
  u = P0 c   (free-axis reduce, VectorE tensor_tensor_reduce)  -> r = 1/u
  t = P0^T r (partition contraction, TensorE vector-stationary) -> AllReduce
and symmetrically for the transposed (text) chain run concurrently on the
other engine. Final CE = one exp pass per side (ScalarE activation fused
row-accumulate; TensorE ones-matmul for column sums) + one AllReduce.
"""

import sys
import numpy as np

if "/opt/trn_rl_repo" not in sys.path:
    sys.path.insert(0, "/opt/trn_rl_repo")

N = 8192
D = 512
CORES = 8
ROWS = N // CORES            # 1024 rows per core
T = ROWS // 128              # 8 row tiles of [128, 8192]
HALF = 2048                  # free-dim strip for VectorE ops
NH = N // HALF               # 4 strips
CH = 512                     # matmul moving chunk
NC = N // CH                 # 16 chunks
MAX_ITERS = 5
B_D = 0.1 * N
B_U = 0.9 * N


def _build(s: float, iters=MAX_ITERS, ce=True, NO_AR=False, DBG=False):
    from concourse import bacc, bass, tile, mybir

    f32 = mybir.dt.float32
    bf16 = mybir.dt.bfloat16
    AF = mybir.ActivationFunctionType
    OP = mybir.AluOpType
    GROUPS = [list(range(CORES))]

    nc = bacc.Bacc("TRN2", target_bir_lowering=False, debug=False,
                   num_devices=CORES)

    imgT_d = nc.dram_tensor("imgT", [D, ROWS], bf16, kind="ExternalInput")
    txtT_d = nc.dram_tensor("txtT", [D, N], bf16, kind="ExternalInput")
    diagp0_d = nc.dram_tensor("diagp0", [128, T], f32, kind="ExternalInput")
    premask_d = nc.dram_tensor("premask", [128, N // 128], f32, kind="ExternalInput")
    ident64_d = nc.dram_tensor("ident64", [64, 64], f32, kind="ExternalInput")
    ones_d = nc.dram_tensor("ones32", [128, 1], f32, kind="ExternalInput")
    onesb_d = nc.dram_tensor("onesb", [128, 1], bf16, kind="ExternalInput")
    out_d = nc.dram_tensor("out", [1, 1], f32, kind="ExternalOutput")
    dbg_d = nc.dram_tensor("dbg", [128, 288], f32, kind="ExternalOutput") if DBG else None

    with tile.TileContext(nc) as tc:
        with tc.tile_pool(name="main", bufs=1) as main, \
             tc.tile_pool(name="dram", bufs=1, space="DRAM") as dpool:

            # ---- long-lived SBUF state ----
            p0 = [main.tile([128, N], bf16, name=f"p0_{i}") for i in range(T)]
            c_rep = main.tile([128, N], bf16, name="c_rep")
            rp_rep = main.tile([128, N], bf16, name="rp_rep")

            u_tl = main.tile([128, T], f32, name="u_tl")
            r_tl = main.tile([128, T], f32, name="r_tl")
            r_bf = main.tile([128, T], bf16, name="r_bf")
            tp_tl = main.tile([128, T], f32, name="tp_tl")
            tpin_tl = main.tile([128, T], f32, name="tpin_tl")
            cp_tl = main.tile([128, T], f32, name="cp_tl")
            cp_bf = main.tile([128, T], bf16, name="cp_bf")

            cY = main.tile([64, N // 64], f32, name="cY")
            c_bfY = main.tile([64, N // 64], bf16, name="c_bfY")
            tY = main.tile([64, N // 64], f32, name="tY")
            upY = main.tile([64, N // 64], f32, name="upY")
            rpY = main.tile([64, N // 64], f32, name="rpY")
            rp_bfY = main.tile([64, N // 64], bf16, name="rp_bfY")

            diagp0 = main.tile([128, T], f32, name="diagp0_sb")
            premask = main.tile([128, N // 128], f32, name="premask_sb")
            ident64 = main.tile([64, 64], f32, name="ident64_sb")
            ones32 = main.tile([128, 1], f32, name="ones32_sb")
            onesb = main.tile([128, 1], bf16, name="onesb_sb")
            zb = main.tile([128, 1], f32, name="zb")
            nc.vector.memset(zb[:], 0.0)
            nb1 = main.tile([128, 1], f32, name="nb1")
            nc.vector.memset(nb1[:], -1.0)

            S16 = main.tile([128, T * NH], f32, name="S16")   # CE partial sumexp
            S8 = main.tile([128, T], f32, name="S8")
            ab = main.tile([128, 4], f32, name="ab")
            SY = main.tile([64, N // 64], f32, name="SY")
            loss_sb = main.tile([1, 1], f32, name="loss_sb")

            # ---- DRAM bounce buffers ----
            ar_in = [dpool.tile([N], f32, name=f"ar_in_{i}") for i in range(2 * MAX_ITERS)]
            ar_out = [dpool.tile([N], f32, name=f"ar_out_{i}") for i in range(2 * MAX_ITERS)]
            ce_in = dpool.tile([N + 2], f32, name="ce_in")
            ce_out = dpool.tile([N + 2], f32, name="ce_out")
            c_lin = dpool.tile([N], bf16, name="c_lin")
            rp_lin = dpool.tile([N], bf16, name="rp_lin")

            # consts in
            nc.sync.dma_start(out=diagp0[:], in_=diagp0_d[:])
            nc.sync.dma_start(out=premask[:], in_=premask_d[:])
            nc.sync.dma_start(out=ident64[:], in_=ident64_d[:])
            nc.sync.dma_start(out=ones32[:], in_=ones_d[:])
            nc.sync.dma_start(out=onesb[:], in_=onesb_d[:])

            # scratch pools for big VectorE/ScalarE strips
            scr_pool = tc.tile_pool(name="scr", bufs=2)
            scr = scr_pool.__enter__()

            # ================= build P0 = exp(s*G - 1) =================
            u_bld = main.tile([128, T * NC], f32, name="u_bld")
            with tc.tile_pool(name="bld", bufs=1) as bld, \
                 tc.tile_pool(name="bldp", bufs=5, space="PSUM") as bldp:
                imgT = [bld.tile([128, ROWS], bf16, name=f"imgT_{k}")
                        for k in range(D // 128)]
                for k in range(D // 128):
                    nc.sync.dma_start(out=imgT[k][:], in_=imgT_d[k * 128:(k + 1) * 128, :])
                for jc in range(NC):
                    txtc = [bld.tile([128, CH], bf16, tag=f"txtc{k}", bufs=2,
                                     name=f"txtc{k}_{jc}")
                            for k in range(D // 128)]
                    for k in range(D // 128):
                        nc.sync.dma_start(out=txtc[k][:],
                                          in_=txtT_d[k * 128:(k + 1) * 128,
                                                     jc * CH:(jc + 1) * CH])
                    psu = bldp.tile([1, CH], f32, tag="psu", bufs=2, name=f"psu_{jc}")
                    for i in range(T):
                        ps = bldp.tile([128, CH], f32, tag="psg", name=f"psg_{jc}_{i}")
                        for k in range(D // 128):
                            nc.tensor.matmul(ps[:], imgT[k][:, i * 128:(i + 1) * 128],
                                             txtc[k][:], start=(k == 0),
                                             stop=(k == D // 128 - 1))
                        nc.scalar.activation(p0[i][:, jc * CH:(jc + 1) * CH], ps[:],
                                             AF.Exp, bias=nb1[:], scale=s,
                                             accum_out=u_bld[:, i * NC + jc:i * NC + jc + 1])
                        nc.tensor.matmul(psu[:], onesb[:],
                                         p0[i][:, jc * CH:(jc + 1) * CH],
                                         start=(i == 0), stop=(i == T - 1))
                    if jc % 2 == 0:
                        vstb = scr.tile([33, 2 * CH], f32, tag="vstg", bufs=2,
                                        name=f"vstb_{jc}")
                    nc.scalar.copy(vstb[0:1, (jc % 2) * CH:(jc % 2 + 1) * CH], psu[:])
                    if jc % 2 == 1:
                        nc.sync.dma_start(
                            out=ar_in[0][(jc - 1) * CH:(jc + 1) * CH],
                            in_=vstb[0:1, :])

            psv_pool = tc.tile_pool(name="psv", bufs=6, space="PSUM")
            psv = psv_pool.__enter__()

            # ---- init sinkhorn state ----
            nc.vector.memset(c_rep[:], 1.0)
            nc.vector.memset(cY[:], 1.0)
            nc.vector.memset(cp_tl[:], 1.0)
            nc.vector.memset(cp_bf[:], 1.0)


            acc16 = main.tile([128, T * NH], f32, name="acc16")

            def mv_free(in1_rep, acc):
                """acc[:, t] = sum_j p0[t][p, j] * in1_rep[p, j] (VectorE)"""
                for t in range(T):
                    for h in range(NH):
                        sc = scr.tile([128, HALF], bf16, tag="scr", bufs=4, name=f"scr_{t}_{h}")
                        nc.vector.scalar_tensor_tensor(
                            out=sc[:], in0=p0[t][:, h * HALF:(h + 1) * HALF],
                            scalar=1.0, in1=in1_rep[:, h * HALF:(h + 1) * HALF],
                            op0=OP.mult, op1=OP.mult,
                            accum_out=acc16[:, t * NH + h:t * NH + h + 1])
                nc.vector.tensor_reduce(acc[:], acc16.rearrange("p (t h) -> p t h", t=T),
                                        axis=mybir.AxisListType.X, op=OP.add)

            def mv_part(stat_bf, it, buf_idx):
                """ar_in[buf_idx] <- sum_i p0[:, :]^T stat (TensorE)"""
                for g in range(NC // 2):
                    vst = scr.tile([33, 2 * CH], f32, tag="vstg", bufs=2,
                                   name=f"vst_{it}_{buf_idx}_{g}")
                    for cw in range(2):
                        cc = g * 2 + cw
                        ps = psv.tile([1, CH], f32, tag="psv", name=f"psv_{it}_{buf_idx}_{cc}")
                        for i in range(T):
                            nc.tensor.matmul(ps[:], stat_bf[:, i:i + 1],
                                             p0[i][:, cc * CH:(cc + 1) * CH],
                                             start=(i == 0), stop=(i == T - 1))
                        nc.scalar.copy(vst[0:1, cw * CH:(cw + 1) * CH], ps[:])
                    nc.sync.dma_start(out=ar_in[buf_idx][g * 2 * CH:(g + 1) * 2 * CH],
                                      in_=vst[0:1, :])

            def allreduce(buf_idx):
                if NO_AR:
                    nc.gpsimd.dma_start(out=ar_out[buf_idx][:], in_=ar_in[buf_idx][:])
                else:
                    nc.gpsimd.collective_compute(
                        "AllReduce", OP.add, replica_groups=GROUPS,
                        ins=[ar_in[buf_idx][:].opt()], outs=[ar_out[buf_idx][:].opt()])

            # ================= sinkhorn iterations =================
            import os
            _PARTS = os.environ.get("KPARTS", "all")
            for it in range(iters):
                # --- phase 1: u = P0 c (V),  u' = P0^T c' (Te) -> AR ---
                if it == 0:
                    # fused into build: u via ACT accum, u' via ones-matmuls
                    nc.vector.tensor_reduce(u_tl[:], u_bld.rearrange("p (t c) -> p t c", t=T),
                                            axis=mybir.AxisListType.X, op=OP.add)
                    allreduce(0)
                    nc.vector.reciprocal(r_tl[:], u_tl[:])
                    nc.scalar.copy(r_bf[:], r_tl[:])
                    nc.sync.dma_start(out=upY[:], in_=ar_out[0].rearrange("(p f) -> p f", p=64))
                    nc.vector.reciprocal(rpY[:], upY[:])
                    nc.scalar.copy(rp_bfY[:], rpY[:])
                elif _PARTS == "vonly":
                    mv_free(c_rep, u_tl)
                    nc.vector.reciprocal(r_tl[:], u_tl[:])
                    continue
                if _PARTS == "tonly":
                    mv_part(cp_bf, it, 2 * it)
                    allreduce(2 * it)
                    nc.sync.dma_start(out=upY[:], in_=ar_out[2 * it].rearrange("(p f) -> p f", p=64))
                    nc.vector.reciprocal(rpY[:], upY[:])
                    nc.vector.tensor_reduce(loss_sb[:], upY[0:1, :], axis=mybir.AxisListType.X, op=OP.add)
                    nc.vector.tensor_copy(u_tl[:], r_tl[:]) if False else None
                    continue
                if _PARTS == "norep":
                    mv_part(cp_bf, it, 2 * it)
                    mv_free(c_rep, u_tl)
                    allreduce(2 * it)
                    nc.vector.reciprocal(r_tl[:], u_tl[:])
                    nc.sync.dma_start(out=upY[:], in_=ar_out[2 * it].rearrange("(p f) -> p f", p=64))
                    nc.vector.reciprocal(rpY[:], upY[:])
                    continue
                else:
                    mv_part(cp_bf, it, 2 * it)
                    mv_free(c_rep, u_tl)
                    allreduce(2 * it)
                    nc.vector.reciprocal(r_tl[:], u_tl[:])
                    nc.scalar.copy(r_bf[:], r_tl[:])
                    # u' back, r' = 1/u', replicate
                    nc.sync.dma_start(out=upY[:], in_=ar_out[2 * it].rearrange("(p f) -> p f", p=64))
                    nc.vector.reciprocal(rpY[:], upY[:])
                    nc.scalar.copy(rp_bfY[:], rpY[:])
                nc.sync.dma_start(out=rp_lin[:], in_=rp_bfY[:])
                for hh in range(NH):
                    nc.sync.dma_start(
                        out=rp_rep[:, hh * HALF:(hh + 1) * HALF],
                        in_=rp_lin[hh * HALF:(hh + 1) * HALF].partition_broadcast(128))

                # --- phase 2: t' = P0 r' (V), t = P0^T r (Te) -> AR ---
                mv_part(r_bf, it, 2 * it + 1)
                mv_free(rp_rep, tp_tl)
                allreduce(2 * it + 1)
                # c' update (local)
                nc.vector.reciprocal(tpin_tl[:], tp_tl[:])
                nc.vector.scalar_tensor_tensor(out=cp_tl[:], in0=tpin_tl[:], scalar=B_D,
                                               in1=cp_tl[:], op0=OP.mult, op1=OP.max)
                nc.vector.scalar_tensor_tensor(out=cp_tl[:], in0=tpin_tl[:], scalar=B_U,
                                               in1=cp_tl[:], op0=OP.mult, op1=OP.min)
                nc.scalar.copy(cp_bf[:], cp_tl[:])
                # c update (Y-form) + replicate
                nc.sync.dma_start(out=tY[:], in_=ar_out[2 * it + 1].rearrange("(p f) -> p f", p=64))
                nc.vector.reciprocal(tY[:], tY[:])
                nc.vector.scalar_tensor_tensor(out=cY[:], in0=tY[:], scalar=B_D,
                                               in1=cY[:], op0=OP.mult, op1=OP.max)
                nc.vector.scalar_tensor_tensor(out=cY[:], in0=tY[:], scalar=B_U,
                                               in1=cY[:], op0=OP.mult, op1=OP.min)
                nc.scalar.copy(c_bfY[:], cY[:])
                nc.sync.dma_start(out=c_lin[:], in_=c_bfY[:])
                for hh in range(NH):
                    nc.sync.dma_start(
                        out=c_rep[:, hh * HALF:(hh + 1) * HALF],
                        in_=c_lin[hh * HALF:(hh + 1) * HALF].partition_broadcast(128))

            psv_pool.__exit__(None, None, None)

            if not ce:
                if iters > 0:
                    nc.vector.tensor_reduce(loss_sb[:], u_tl[0:1, :],
                                            axis=mybir.AxisListType.X, op=OP.add)
                else:
                    nc.vector.tensor_reduce(loss_sb[:], p0[0][0:1, 0:128],
                                            axis=mybir.AxisListType.X, op=OP.add)
                nc.sync.dma_start(out=out_d[:], in_=loss_sb[:])
            # ================= cross-entropy =================
            # image side: S_i = sum_j exp(r_i P0_ij c_j); text: S'_j = sum_i exp(r'_j P0_ij c'_i)
            if not ce:
                pass
            else:
              with tc.tile_pool(name="cep", bufs=2, space="PSUM") as cep:
                # transposes for diag extraction: f-major [128, 64] forms
                ps_cfm = cep.tile([128, 64], f32, tag="tp", name="ps_cfm")
                nc.tensor.transpose(ps_cfm[:], cY[:], ident64[:])
                c_fm = main.tile([128, 64], f32, name="c_fm")
                nc.scalar.copy(c_fm[:], ps_cfm[:])
                ps_rfm = cep.tile([128, 64], f32, tag="tp", name="ps_rfm")
                nc.tensor.transpose(ps_rfm[:], rpY[:], ident64[:])
                rp_fm = main.tile([128, 64], f32, name="rp_fm")
                nc.scalar.copy(rp_fm[:], ps_rfm[:])

              with tc.tile_pool(name="cesp", bufs=1, space="PSUM") as cesp:
                ps_sp = cesp.tile([64, N // 2], f32, name="ps_sp")
                # text side first: depends only on rp_rep/cp_tl (ready before
                # iteration 5's c-chain finishes) so ScalarE's in-order FIFO
                # isn't blocked by image ops waiting on c_rep.
                for t in range(T):
                    for h in range(NH):
                        pr2 = scr.tile([128, HALF], bf16, tag="scr", bufs=4, name=f"prt_{t}_{h}")
                        nc.vector.scalar_tensor_tensor(
                            out=pr2[:], in0=p0[t][:, h * HALF:(h + 1) * HALF],
                            scalar=0.0, in1=rp_rep[:, h * HALF:(h + 1) * HALF],
                            op0=OP.add, op1=OP.mult)
                        es = scr.tile([128, HALF], bf16, tag="scr", bufs=4, name=f"es_{t}_{h}")
                        nc.scalar.activation(es[:], pr2[:], AF.Exp, bias=zb[:],
                                             scale=cp_tl[:, t:t + 1])
                        for cw in range(HALF // CH):
                            cc = h * (HALF // CH) + cw
                            pp = 0 if cc < NC // 2 else 32
                            cm = cc % (NC // 2)
                            nc.tensor.matmul(
                                ps_sp[pp:pp + 1, cm * CH:(cm + 1) * CH],
                                onesb[:], es[:, cw * CH:(cw + 1) * CH],
                                start=(t == 0), stop=(t == T - 1))
                for t in range(T):
                    for h in range(NH):
                        pr = scr.tile([128, HALF], bf16, tag="scr", bufs=4, name=f"pri_{t}_{h}")
                        nc.vector.scalar_tensor_tensor(
                            out=pr[:], in0=p0[t][:, h * HALF:(h + 1) * HALF],
                            scalar=0.0, in1=c_rep[:, h * HALF:(h + 1) * HALF],
                            op0=OP.add, op1=OP.mult)
                        go = scr.tile([128, HALF], bf16, tag="scr", bufs=4, name=f"go_{t}_{h}")
                        nc.scalar.activation(go[:], pr[:], AF.Exp, bias=zb[:],
                                             scale=r_tl[:, t:t + 1],
                                             accum_out=S16[:, t * NH + h:t * NH + h + 1])

                # S per row = sum of half-sums, then log, then row-reduce
                nc.vector.tensor_reduce(S8[:], S16.rearrange("p (t h) -> p t h", t=T),
                                        axis=mybir.AxisListType.X, op=OP.add)
                logS = main.tile([128, T], f32, name="logS")
                nc.scalar.activation(logS[:], S8[:], AF.Ln, bias=zb[:])
                nc.vector.tensor_reduce(ab[:, 0:1], logS[:], axis=mybir.AxisListType.X,
                                        op=OP.add)

                # diagonal terms via premask trick
                v1 = main.tile([128, T], f32, name="v1")
                nc.vector.scalar_tensor_tensor(out=v1[:], in0=r_tl[:], scalar=0.0,
                                               in1=diagp0[:], op0=OP.add, op1=OP.mult)
                v1r = main.tile([128, 64], f32, name="v1r")
                for h8 in range(64 // T):
                    nc.vector.tensor_copy(v1r[:, h8 * T:(h8 + 1) * T], v1[:])
                cfm_m = main.tile([128, 64], f32, name="cfm_m")
                nc.vector.scalar_tensor_tensor(out=cfm_m[:], in0=c_fm[:], scalar=0.0,
                                               in1=premask[:], op0=OP.add, op1=OP.mult)
                g1 = main.tile([128, 64], f32, name="g1")
                nc.vector.scalar_tensor_tensor(out=g1[:], in0=v1r[:], scalar=1.0,
                                               in1=cfm_m[:], op0=OP.mult, op1=OP.mult,
                                               accum_out=ab[:, 1:2])
                v2 = main.tile([128, T], f32, name="v2")
                nc.vector.scalar_tensor_tensor(out=v2[:], in0=cp_tl[:], scalar=0.0,
                                               in1=diagp0[:], op0=OP.add, op1=OP.mult)
                v2r = main.tile([128, 64], f32, name="v2r")
                for h8 in range(64 // T):
                    nc.vector.tensor_copy(v2r[:, h8 * T:(h8 + 1) * T], v2[:])
                rfm_m = main.tile([128, 64], f32, name="rfm_m")
                nc.vector.scalar_tensor_tensor(out=rfm_m[:], in0=rp_fm[:], scalar=0.0,
                                               in1=premask[:], op0=OP.add, op1=OP.mult)
                g2 = main.tile([128, 64], f32, name="g2")
                nc.vector.scalar_tensor_tensor(out=g2[:], in0=v2r[:], scalar=1.0,
                                               in1=rfm_m[:], op0=OP.mult, op1=OP.mult,
                                               accum_out=ab[:, 2:3])
                nc.vector.memset(ab[:, 3:4], 0.0)

                # ship S' partial + (a=sum logS - diag_img, b=diag_txt) in one AR
                for half in range(2):
                    pp = 0 if half == 0 else 32
                    for g in range(4):
                        vsa = scr.tile([33, 2 * CH], f32, tag="vstg", bufs=2,
                                       name=f"vsa_{half}_{g}")
                        nc.scalar.copy(vsa[pp:pp + 1, :],
                                       ps_sp[pp:pp + 1, g * 2 * CH:(g + 1) * 2 * CH])
                        nc.sync.dma_start(
                            out=ce_in[half * (N // 2) + g * 2 * CH:
                                      half * (N // 2) + (g + 1) * 2 * CH],
                            in_=vsa[pp:pp + 1, :])

              with tc.tile_pool(name="ceab", bufs=1, space="PSUM") as ceab:
                ps_ab = ceab.tile([1, 4], f32, name="ps_ab")
                nc.tensor.matmul(ps_ab[:], ones32[:], ab[:], start=True, stop=True)
                ab_row = main.tile([1, 4], f32, name="ab_row")
                nc.scalar.copy(ab_row[:], ps_ab[:])
                a_sb = main.tile([1, 2], f32, name="a_sb")
                nc.vector.scalar_tensor_tensor(out=a_sb[0:1, 0:1], in0=ab_row[0:1, 0:1],
                                               scalar=0.0, in1=ab_row[0:1, 1:2],
                                               op0=OP.add, op1=OP.subtract)
                nc.vector.tensor_copy(a_sb[0:1, 1:2], ab_row[0:1, 2:3])
                nc.sync.dma_start(out=ce_in[N:N + 2], in_=a_sb[0:1, :])

                nc.gpsimd.collective_compute(
                    "AllReduce", OP.add, replica_groups=GROUPS,
                    ins=[ce_in[:].opt()], outs=[ce_out[:].opt()])

                # LT = sum_j log S'_j ; loss = (a + LT - b) / (2N)
                nc.sync.dma_start(out=SY[:], in_=ce_out[0:N].rearrange("(p f) -> p f", p=64))
                ab_fin = main.tile([1, 2], f32, name="ab_fin")
                nc.sync.dma_start(out=ab_fin[0:1, :], in_=ce_out[N:N + 2])
                gY = main.tile([64, N // 64], f32, name="gY")
                ltY = main.tile([64, 1], f32, name="ltY")
                nc.scalar.activation(gY[:], SY[:], AF.Ln, bias=zb[0:64, :], accum_out=ltY[:])
                ps_lt = ceab.tile([1, 1], f32, name="ps_lt")
                nc.tensor.matmul(ps_lt[:], ones32[0:64, :], ltY[:], start=True, stop=True)
                lt_row = main.tile([1, 1], f32, name="lt_row")
                nc.scalar.copy(lt_row[:], ps_lt[:])
                nc.vector.scalar_tensor_tensor(out=loss_sb[:], in0=ab_fin[0:1, 0:1],
                                               scalar=0.0, in1=lt_row[:],
                                               op0=OP.add, op1=OP.add)
                nc.vector.scalar_tensor_tensor(out=loss_sb[:], in0=loss_sb[:],
                                               scalar=0.0, in1=ab_fin[0:1, 1:2],
                                               op0=OP.add, op1=OP.subtract)
                nc.scalar.mul(loss_sb[:], loss_sb[:], 1.0 / (2.0 * N))
                nc.sync.dma_start(out=out_d[:], in_=loss_sb[:])
                dbg = main.tile([128, 288], f32, name="dbg") if DBG else None
                if DBG:
                    nc.vector.memset(dbg[:], 0.0)
                    nc.vector.tensor_copy(dbg[:, 0:T], r_tl[:])
                    nc.vector.tensor_copy(dbg[:, 8:8 + T], cp_tl[:])
                    nc.vector.tensor_copy(dbg[0:64, 16:16 + N // 64], cY[:])
                    nc.vector.tensor_copy(dbg[0:64, 144:144 + N // 64], rpY[:])
                    nc.vector.tensor_copy(dbg[:, 272:272 + T], S8[:])
                    nc.vector.tensor_copy(dbg[:, 280:284], ab[:])
                    nc.vector.tensor_copy(dbg[0:1, 284:288], ab_row[:])
                    nc.vector.tensor_copy(dbg[0:1, 270:271], lt_row[:])
                    nc.sync.dma_start(out=dbg_d[:], in_=dbg[:])

            scr_pool.__exit__(None, None, None)

    nc.finalize()
    return nc


def kernel(all_image_features, all_text_features, logit_scale, labels=None,
           **kwargs):
    import ml_dtypes
    from concourse.bass_utils import run_bass_kernel_spmd

    s = float(np.asarray(logit_scale))
    img = np.asarray(all_image_features, np.float32)
    txt = np.asarray(all_text_features, np.float32)

    bf = ml_dtypes.bfloat16
    imgb = img.astype(bf)
    txtb = txt.astype(bf)
    txtT = np.ascontiguousarray(txtb.T)

    # host-side diag of P0 (from the bf16-cast features, like the device path)
    dg = np.exp(s * np.einsum("nd,nd->n", imgb.astype(np.float32),
                              txtb.astype(np.float32)) - 1.0).astype(np.float32)

    ident64 = np.eye(64, dtype=np.float32)
    ones32 = np.ones([128, 1], np.float32)
    onesb = np.ones([128, 1], bf)

    in_maps = []
    for k in range(CORES):
        sl = slice(k * ROWS, (k + 1) * ROWS)
        imgT_k = np.ascontiguousarray(imgb[sl].T)
        diag_k = np.ascontiguousarray(dg[sl].reshape(T, 128).T)   # [p, t]
        premask_k = np.zeros([128, N // 128], np.float32)
        premask_k[:, k * T:(k + 1) * T] = 1.0
        in_maps.append({
            "imgT": imgT_k, "txtT": txtT, "diagp0": diag_k,
            "premask": premask_k, "ident64": ident64,
            "ones32": ones32, "onesb": onesb,
        })

    global LAST_NC, LAST_IN_MAPS, LAST_RESULTS, _NC_KEY
    key = (s, int(kwargs.get('_iters', MAX_ITERS)), bool(kwargs.get('_ce', True)),
           bool(kwargs.get('_noar', False)))
    if globals().get('_NC_KEY') == key and globals().get('LAST_NC') is not None:
        nc = LAST_NC
    else:
        nc = _build(s, iters=key[1], ce=key[2], NO_AR=key[3])
        _NC_KEY = key
    LAST_NC, LAST_IN_MAPS = nc, in_maps
    res = run_bass_kernel_spmd(nc, in_maps, list(range(CORES)))
    LAST_RESULTS = res
    return np.float32(res.results[0]["out"][0, 0])


if __name__ == "__main__":
    import reference
    inputs = reference.setup_inputs()
    out = kernel(**{k: np.asarray(v) for k, v in inputs.items()})
    print("kernel loss:", out)



# revision 8
# speedup vs baseline: 1.4976x; 1.4976x over previous
"""DBOT Sinkhorn loss kernel for 8 TRN2 NeuronCores.

Strategy: P0 = exp(s*img@txt.T - 1) is row-sharded (1024 rows/core) and kept
SBUF-resident in bf16 (16MB/core). Sinkhorn iterations factor exactly as
P = diag(r) P0 diag(c), so each iteration is two matvec passes:
  u = P0 c   (free-axis reduce, VectorE tensor_tensor_reduce)  -> r = 1/u
  t = P0^T r (partition contraction, TensorE vector-stationary) -> AllReduce
and symmetrically for the transposed (text) chain run concurrently on the
other engine. Final CE = one exp pass per side (ScalarE activation fused
row-accumulate; TensorE ones-matmul for column sums) + one AllReduce.
"""

import sys
import numpy as np

if "/opt/trn_rl_repo" not in sys.path:
    sys.path.insert(0, "/opt/trn_rl_repo")

N = 8192
D = 512
CORES = 8
ROWS = N // CORES            # 1024 rows per core
T = ROWS // 128              # 8 row tiles of [128, 8192]
HALF = 2048                  # free-dim strip for VectorE ops
NH = N // HALF               # 4 strips
CH = 512                     # matmul moving chunk
NC = N // CH                 # 16 chunks
MAX_ITERS = 5
B_D = 0.1 * N
B_U = 0.9 * N


def _build(s: float, iters=MAX_ITERS, ce=True, NO_AR=False, DBG=False):
    from concourse import bacc, bass, tile, mybir

    f32 = mybir.dt.float32
    bf16 = mybir.dt.bfloat16
    AF = mybir.ActivationFunctionType
    OP = mybir.AluOpType
    GROUPS = [list(range(CORES))]

    nc = bacc.Bacc("TRN2", target_bir_lowering=False, debug=False,
                   num_devices=CORES)

    imgT_d = nc.dram_tensor("imgT", [D, ROWS], bf16, kind="ExternalInput")
    # per-core slice of txt^T, packed as [2, D, CH] halves; allgathered on
    # device so only 1MB/core ships per call instead of 8MB replicated.
    txtTs_d = nc.dram_tensor("txtTs", [2, D, CH], bf16, kind="ExternalInput")
    diagp0_d = nc.dram_tensor("diagp0", [128, T], f32, kind="ExternalInput")
    premask_d = nc.dram_tensor("premask", [128, N // 128], f32, kind="ExternalInput")
    ident64_d = nc.dram_tensor("ident64", [64, 64], f32, kind="ExternalInput")
    ones_d = nc.dram_tensor("ones32", [128, 1], f32, kind="ExternalInput")
    onesb_d = nc.dram_tensor("onesb", [128, 1], bf16, kind="ExternalInput")
    out_d = nc.dram_tensor("out", [1, 1], f32, kind="ExternalOutput")
    dbg_d = nc.dram_tensor("dbg", [128, 288], f32, kind="ExternalOutput") if DBG else None

    with tile.TileContext(nc) as tc:
        with tc.tile_pool(name="main", bufs=1) as main, \
             tc.tile_pool(name="dram", bufs=1, space="DRAM") as dpool:

            # ---- long-lived SBUF state ----
            p0 = [main.tile([128, N], bf16, name=f"p0_{i}") for i in range(T)]
            c_rep = main.tile([128, N], bf16, name="c_rep")
            rp_rep = main.tile([128, N], bf16, name="rp_rep")

            u_tl = main.tile([128, T], f32, name="u_tl")
            r_tl = main.tile([128, T], f32, name="r_tl")
            r_bf = main.tile([128, T], bf16, name="r_bf")
            tp_tl = main.tile([128, T], f32, name="tp_tl")
            tpin_tl = main.tile([128, T], f32, name="tpin_tl")
            cp_tl = main.tile([128, T], f32, name="cp_tl")
            cp_bf = main.tile([128, T], bf16, name="cp_bf")

            cY = main.tile([64, N // 64], f32, name="cY")
            c_bfY = main.tile([64, N // 64], bf16, name="c_bfY")
            tY = main.tile([64, N // 64], f32, name="tY")
            upY = main.tile([64, N // 64], f32, name="upY")
            rpY = main.tile([64, N // 64], f32, name="rpY")
            rp_bfY = main.tile([64, N // 64], bf16, name="rp_bfY")

            diagp0 = main.tile([128, T], f32, name="diagp0_sb")
            premask = main.tile([128, N // 128], f32, name="premask_sb")
            ident64 = main.tile([64, 64], f32, name="ident64_sb")
            ones32 = main.tile([128, 1], f32, name="ones32_sb")
            onesb = main.tile([128, 1], bf16, name="onesb_sb")
            zb = main.tile([128, 1], f32, name="zb")
            nc.vector.memset(zb[:], 0.0)
            nb1 = main.tile([128, 1], f32, name="nb1")
            nc.vector.memset(nb1[:], -1.0)

            S16 = main.tile([128, T * NH], f32, name="S16")   # CE partial sumexp
            S8 = main.tile([128, T], f32, name="S8")
            ab = main.tile([128, 4], f32, name="ab")
            SY = main.tile([64, N // 64], f32, name="SY")
            loss_sb = main.tile([1, 1], f32, name="loss_sb")

            # ---- DRAM bounce buffers ----
            ar_in = [dpool.tile([N], f32, name=f"ar_in_{i}") for i in range(2 * MAX_ITERS)]
            ar_out = [dpool.tile([N], f32, name=f"ar_out_{i}") for i in range(2 * MAX_ITERS)]
            ce_in = dpool.tile([N + 2], f32, name="ce_in")
            ce_out = dpool.tile([N + 2], f32, name="ce_out")
            c_lin = dpool.tile([N], bf16, name="c_lin")
            rp_lin = dpool.tile([N], bf16, name="rp_lin")

            # consts in
            nc.sync.dma_start(out=diagp0[:], in_=diagp0_d[:])
            nc.sync.dma_start(out=premask[:], in_=premask_d[:])
            nc.sync.dma_start(out=ident64[:], in_=ident64_d[:])
            nc.sync.dma_start(out=ones32[:], in_=ones_d[:])
            nc.sync.dma_start(out=onesb[:], in_=onesb_d[:])

            # scratch pools for big VectorE/ScalarE strips
            scr_pool = tc.tile_pool(name="scr", bufs=2)
            scr = scr_pool.__enter__()

            # ================= build P0 = exp(s*G - 1) =================
            # txt^T arrives sharded (1MB/core); allgather it on device in two
            # halves so PE work on half 0 overlaps the gather of half 1.
            txt_ag = [dpool.tile([CORES, D, CH], bf16, addr_space="Shared",
                                 name=f"txt_ag{h}") for h in range(2)]
            txt_bnc = dpool.tile([2, D, CH], bf16, name="txt_bnc")
            for h in range(2):
                # collectives can't read IO tensors; bounce through internal DRAM
                nc.sync.dma_start(out=txt_bnc[h], in_=txtTs_d[h])
                nc.gpsimd.collective_compute(
                    "AllGather", OP.bypass, replica_groups=GROUPS,
                    ins=[txt_bnc[h].opt()], outs=[txt_ag[h][:].opt()])
            u_bld = main.tile([128, T * NC], f32, name="u_bld")
            with tc.tile_pool(name="bld", bufs=1) as bld, \
                 tc.tile_pool(name="bldp", bufs=5, space="PSUM") as bldp:
                imgT = [bld.tile([128, ROWS], bf16, name=f"imgT_{k}")
                        for k in range(D // 128)]
                for k in range(D // 128):
                    nc.sync.dma_start(out=imgT[k][:], in_=imgT_d[k * 128:(k + 1) * 128, :])
                for jc in [2 * g + h for h in range(2) for g in range(NC // 2)]:
                    txtc = [bld.tile([128, CH], bf16, tag=f"txtc{k}", bufs=2,
                                     name=f"txtc{k}_{jc}")
                            for k in range(D // 128)]
                    for k in range(D // 128):
                        nc.sync.dma_start(out=txtc[k][:],
                                          in_=txt_ag[jc % 2][jc // 2,
                                                            k * 128:(k + 1) * 128, :])
                    psu = bldp.tile([1, CH], f32, tag="psu", bufs=2, name=f"psu_{jc}")
                    for i in range(T):
                        ps = bldp.tile([128, CH], f32, tag="psg", name=f"psg_{jc}_{i}")
                        for k in range(D // 128):
                            nc.tensor.matmul(ps[:], imgT[k][:, i * 128:(i + 1) * 128],
                                             txtc[k][:], start=(k == 0),
                                             stop=(k == D // 128 - 1))
                        nc.scalar.activation(p0[i][:, jc * CH:(jc + 1) * CH], ps[:],
                                             AF.Exp, bias=nb1[:], scale=s,
                                             accum_out=u_bld[:, i * NC + jc:i * NC + jc + 1])
                        nc.tensor.matmul(psu[:], onesb[:],
                                         p0[i][:, jc * CH:(jc + 1) * CH],
                                         start=(i == 0), stop=(i == T - 1))
                    vstb = scr.tile([33, 2 * CH], f32, tag="vstg", bufs=2,
                                    name=f"vstb_{jc}")
                    nc.scalar.copy(vstb[0:1, 0:CH], psu[:])
                    nc.sync.dma_start(out=ar_in[0][jc * CH:(jc + 1) * CH],
                                      in_=vstb[0:1, 0:CH])

            psv_pool = tc.tile_pool(name="psv", bufs=6, space="PSUM")
            psv = psv_pool.__enter__()

            # ---- init sinkhorn state ----
            nc.vector.memset(c_rep[:], 1.0)
            nc.vector.memset(cY[:], 1.0)
            nc.vector.memset(cp_tl[:], 1.0)
            nc.vector.memset(cp_bf[:], 1.0)


            acc16 = main.tile([128, T * NH], f32, name="acc16")

            def mv_free(in1_rep, acc):
                """acc[:, t] = sum_j p0[t][p, j] * in1_rep[p, j] (VectorE)"""
                for t in range(T):
                    for h in range(NH):
                        sc = scr.tile([128, HALF], bf16, tag="scr", bufs=4, name=f"scr_{t}_{h}")
                        nc.vector.scalar_tensor_tensor(
                            out=sc[:], in0=p0[t][:, h * HALF:(h + 1) * HALF],
                            scalar=1.0, in1=in1_rep[:, h * HALF:(h + 1) * HALF],
                            op0=OP.mult, op1=OP.mult,
                            accum_out=acc16[:, t * NH + h:t * NH + h + 1])
                nc.vector.tensor_reduce(acc[:], acc16.rearrange("p (t h) -> p t h", t=T),
                                        axis=mybir.AxisListType.X, op=OP.add)

            def mv_part(stat_bf, it, buf_idx):
                """ar_in[buf_idx] <- sum_i p0[:, :]^T stat (TensorE)"""
                for g in range(NC // 2):
                    vst = scr.tile([33, 2 * CH], f32, tag="vstg", bufs=2,
                                   name=f"vst_{it}_{buf_idx}_{g}")
                    for cw in range(2):
                        cc = g * 2 + cw
                        ps = psv.tile([1, CH], f32, tag="psv", name=f"psv_{it}_{buf_idx}_{cc}")
                        for i in range(T):
                            nc.tensor.matmul(ps[:], stat_bf[:, i:i + 1],
                                             p0[i][:, cc * CH:(cc + 1) * CH],
                                             start=(i == 0), stop=(i == T - 1))
                        nc.scalar.copy(vst[0:1, cw * CH:(cw + 1) * CH], ps[:])
                    nc.sync.dma_start(out=ar_in[buf_idx][g * 2 * CH:(g + 1) * 2 * CH],
                                      in_=vst[0:1, :])

            def allreduce(buf_idx):
                if NO_AR:
                    nc.gpsimd.dma_start(out=ar_out[buf_idx][:], in_=ar_in[buf_idx][:])
                else:
                    nc.gpsimd.collective_compute(
                        "AllReduce", OP.add, replica_groups=GROUPS,
                        ins=[ar_in[buf_idx][:].opt()], outs=[ar_out[buf_idx][:].opt()])

            # ================= sinkhorn iterations =================
            import os
            _PARTS = os.environ.get("KPARTS", "all")
            for it in range(iters):
                # --- phase 1: u = P0 c (V),  u' = P0^T c' (Te) -> AR ---
                if it == 0:
                    # fused into build: u via ACT accum, u' via ones-matmuls
                    nc.vector.tensor_reduce(u_tl[:], u_bld.rearrange("p (t c) -> p t c", t=T),
                                            axis=mybir.AxisListType.X, op=OP.add)
                    allreduce(0)
                    nc.vector.reciprocal(r_tl[:], u_tl[:])
                    nc.scalar.copy(r_bf[:], r_tl[:])
                    nc.sync.dma_start(out=upY[:], in_=ar_out[0].rearrange("(p f) -> p f", p=64))
                    nc.vector.reciprocal(rpY[:], upY[:])
                    nc.scalar.copy(rp_bfY[:], rpY[:])
                elif _PARTS == "vonly":
                    mv_free(c_rep, u_tl)
                    nc.vector.reciprocal(r_tl[:], u_tl[:])
                    continue
                if _PARTS == "tonly":
                    mv_part(cp_bf, it, 2 * it)
                    allreduce(2 * it)
                    nc.sync.dma_start(out=upY[:], in_=ar_out[2 * it].rearrange("(p f) -> p f", p=64))
                    nc.vector.reciprocal(rpY[:], upY[:])
                    nc.vector.tensor_reduce(loss_sb[:], upY[0:1, :], axis=mybir.AxisListType.X, op=OP.add)
                    nc.vector.tensor_copy(u_tl[:], r_tl[:]) if False else None
                    continue
                if _PARTS == "norep":
                    mv_part(cp_bf, it, 2 * it)
                    mv_free(c_rep, u_tl)
                    allreduce(2 * it)
                    nc.vector.reciprocal(r_tl[:], u_tl[:])
                    nc.sync.dma_start(out=upY[:], in_=ar_out[2 * it].rearrange("(p f) -> p f", p=64))
                    nc.vector.reciprocal(rpY[:], upY[:])
                    continue
                else:
                    mv_part(cp_bf, it, 2 * it)
                    mv_free(c_rep, u_tl)
                    allreduce(2 * it)
                    nc.vector.reciprocal(r_tl[:], u_tl[:])
                    nc.scalar.copy(r_bf[:], r_tl[:])
                    # u' back, r' = 1/u', replicate
                    nc.sync.dma_start(out=upY[:], in_=ar_out[2 * it].rearrange("(p f) -> p f", p=64))
                    nc.vector.reciprocal(rpY[:], upY[:])
                    nc.scalar.copy(rp_bfY[:], rpY[:])
                nc.sync.dma_start(out=rp_lin[:], in_=rp_bfY[:])
                for hh in range(NH):
                    nc.sync.dma_start(
                        out=rp_rep[:, hh * HALF:(hh + 1) * HALF],
                        in_=rp_lin[hh * HALF:(hh + 1) * HALF].partition_broadcast(128))

                # --- phase 2: t' = P0 r' (V), t = P0^T r (Te) -> AR ---
                mv_part(r_bf, it, 2 * it + 1)
                mv_free(rp_rep, tp_tl)
                allreduce(2 * it + 1)
                # c' update (local)
                nc.vector.reciprocal(tpin_tl[:], tp_tl[:])
                nc.vector.scalar_tensor_tensor(out=cp_tl[:], in0=tpin_tl[:], scalar=B_D,
                                               in1=cp_tl[:], op0=OP.mult, op1=OP.max)
                nc.vector.scalar_tensor_tensor(out=cp_tl[:], in0=tpin_tl[:], scalar=B_U,
                                               in1=cp_tl[:], op0=OP.mult, op1=OP.min)
                nc.scalar.copy(cp_bf[:], cp_tl[:])
                # c update (Y-form) + replicate
                nc.sync.dma_start(out=tY[:], in_=ar_out[2 * it + 1].rearrange("(p f) -> p f", p=64))
                nc.vector.reciprocal(tY[:], tY[:])
                nc.vector.scalar_tensor_tensor(out=cY[:], in0=tY[:], scalar=B_D,
                                               in1=cY[:], op0=OP.mult, op1=OP.max)
                nc.vector.scalar_tensor_tensor(out=cY[:], in0=tY[:], scalar=B_U,
                                               in1=cY[:], op0=OP.mult, op1=OP.min)
                nc.scalar.copy(c_bfY[:], cY[:])
                nc.sync.dma_start(out=c_lin[:], in_=c_bfY[:])
                for hh in range(NH):
                    nc.sync.dma_start(
                        out=c_rep[:, hh * HALF:(hh + 1) * HALF],
                        in_=c_lin[hh * HALF:(hh + 1) * HALF].partition_broadcast(128))

            psv_pool.__exit__(None, None, None)

            if not ce:
                if iters > 0:
                    nc.vector.tensor_reduce(loss_sb[:], u_tl[0:1, :],
                                            axis=mybir.AxisListType.X, op=OP.add)
                else:
                    nc.vector.tensor_reduce(loss_sb[:], p0[0][0:1, 0:128],
                                            axis=mybir.AxisListType.X, op=OP.add)
                nc.sync.dma_start(out=out_d[:], in_=loss_sb[:])
            # ================= cross-entropy =================
            # image side: S_i = sum_j exp(r_i P0_ij c_j); text: S'_j = sum_i exp(r'_j P0_ij c'_i)
            if not ce:
                pass
            else:
              with tc.tile_pool(name="cep", bufs=2, space="PSUM") as cep:
                # transposes for diag extraction: f-major [128, 64] forms
                ps_cfm = cep.tile([128, 64], f32, tag="tp", name="ps_cfm")
                nc.tensor.transpose(ps_cfm[:], cY[:], ident64[:])
                c_fm = main.tile([128, 64], f32, name="c_fm")
                nc.scalar.copy(c_fm[:], ps_cfm[:])
                ps_rfm = cep.tile([128, 64], f32, tag="tp", name="ps_rfm")
                nc.tensor.transpose(ps_rfm[:], rpY[:], ident64[:])
                rp_fm = main.tile([128, 64], f32, name="rp_fm")
                nc.scalar.copy(rp_fm[:], ps_rfm[:])

              with tc.tile_pool(name="cesp", bufs=1, space="PSUM") as cesp:
                ps_sp = cesp.tile([64, N // 2], f32, name="ps_sp")
                # text side first: depends only on rp_rep/cp_tl (ready before
                # iteration 5's c-chain finishes) so ScalarE's in-order FIFO
                # isn't blocked by image ops waiting on c_rep.
                for t in range(T):
                    for h in range(NH):
                        pr2 = scr.tile([128, HALF], bf16, tag="scr", bufs=4, name=f"prt_{t}_{h}")
                        nc.vector.scalar_tensor_tensor(
                            out=pr2[:], in0=p0[t][:, h * HALF:(h + 1) * HALF],
                            scalar=0.0, in1=rp_rep[:, h * HALF:(h + 1) * HALF],
                            op0=OP.add, op1=OP.mult)
                        es = scr.tile([128, HALF], bf16, tag="scr", bufs=4, name=f"es_{t}_{h}")
                        nc.scalar.activation(es[:], pr2[:], AF.Exp, bias=zb[:],
                                             scale=cp_tl[:, t:t + 1])
                        for cw in range(HALF // CH):
                            cc = h * (HALF // CH) + cw
                            pp = 0 if cc < NC // 2 else 32
                            cm = cc % (NC // 2)
                            nc.tensor.matmul(
                                ps_sp[pp:pp + 1, cm * CH:(cm + 1) * CH],
                                onesb[:], es[:, cw * CH:(cw + 1) * CH],
                                start=(t == 0), stop=(t == T - 1))
                for t in range(T):
                    for h in range(NH):
                        pr = scr.tile([128, HALF], bf16, tag="scr", bufs=4, name=f"pri_{t}_{h}")
                        nc.vector.scalar_tensor_tensor(
                            out=pr[:], in0=p0[t][:, h * HALF:(h + 1) * HALF],
                            scalar=0.0, in1=c_rep[:, h * HALF:(h + 1) * HALF],
                            op0=OP.add, op1=OP.mult)
                        go = scr.tile([128, HALF], bf16, tag="scr", bufs=4, name=f"go_{t}_{h}")
                        nc.scalar.activation(go[:], pr[:], AF.Exp, bias=zb[:],
                                             scale=r_tl[:, t:t + 1],
                                             accum_out=S16[:, t * NH + h:t * NH + h + 1])

                # S per row = sum of half-sums, then log, then row-reduce
                nc.vector.tensor_reduce(S8[:], S16.rearrange("p (t h) -> p t h", t=T),
                                        axis=mybir.AxisListType.X, op=OP.add)
                logS = main.tile([128, T], f32, name="logS")
                nc.scalar.activation(logS[:], S8[:], AF.Ln, bias=zb[:])
                nc.vector.tensor_reduce(ab[:, 0:1], logS[:], axis=mybir.AxisListType.X,
                                        op=OP.add)

                # diagonal terms via premask trick
                v1 = main.tile([128, T], f32, name="v1")
                nc.vector.scalar_tensor_tensor(out=v1[:], in0=r_tl[:], scalar=0.0,
                                               in1=diagp0[:], op0=OP.add, op1=OP.mult)
                v1r = main.tile([128, 64], f32, name="v1r")
                for h8 in range(64 // T):
                    nc.vector.tensor_copy(v1r[:, h8 * T:(h8 + 1) * T], v1[:])
                cfm_m = main.tile([128, 64], f32, name="cfm_m")
                nc.vector.scalar_tensor_tensor(out=cfm_m[:], in0=c_fm[:], scalar=0.0,
                                               in1=premask[:], op0=OP.add, op1=OP.mult)
                g1 = main.tile([128, 64], f32, name="g1")
                nc.vector.scalar_tensor_tensor(out=g1[:], in0=v1r[:], scalar=1.0,
                                               in1=cfm_m[:], op0=OP.mult, op1=OP.mult,
                                               accum_out=ab[:, 1:2])
                v2 = main.tile([128, T], f32, name="v2")
                nc.vector.scalar_tensor_tensor(out=v2[:], in0=cp_tl[:], scalar=0.0,
                                               in1=diagp0[:], op0=OP.add, op1=OP.mult)
                v2r = main.tile([128, 64], f32, name="v2r")
                for h8 in range(64 // T):
                    nc.vector.tensor_copy(v2r[:, h8 * T:(h8 + 1) * T], v2[:])
                rfm_m = main.tile([128, 64], f32, name="rfm_m")
                nc.vector.scalar_tensor_tensor(out=rfm_m[:], in0=rp_fm[:], scalar=0.0,
                                               in1=premask[:], op0=OP.add, op1=OP.mult)
                g2 = main.tile([128, 64], f32, name="g2")
                nc.vector.scalar_tensor_tensor(out=g2[:], in0=v2r[:], scalar=1.0,
                                               in1=rfm_m[:], op0=OP.mult, op1=OP.mult,
                                               accum_out=ab[:, 2:3])
                nc.vector.memset(ab[:, 3:4], 0.0)

                # ship S' partial + (a=sum logS - diag_img, b=diag_txt) in one AR
                for half in range(2):
                    pp = 0 if half == 0 else 32
                    for g in range(4):
                        vsa = scr.tile([33, 2 * CH], f32, tag="vstg", bufs=2,
                                       name=f"vsa_{half}_{g}")
                        nc.scalar.copy(vsa[pp:pp + 1, :],
                                       ps_sp[pp:pp + 1, g * 2 * CH:(g + 1) * 2 * CH])
                        nc.sync.dma_start(
                            out=ce_in[half * (N // 2) + g * 2 * CH:
                                      half * (N // 2) + (g + 1) * 2 * CH],
                            in_=vsa[pp:pp + 1, :])

              with tc.tile_pool(name="ceab", bufs=1, space="PSUM") as ceab:
                ps_ab = ceab.tile([1, 4], f32, name="ps_ab")
                nc.tensor.matmul(ps_ab[:], ones32[:], ab[:], start=True, stop=True)
                ab_row = main.tile([1, 4], f32, name="ab_row")
                nc.scalar.copy(ab_row[:], ps_ab[:])
                a_sb = main.tile([1, 2], f32, name="a_sb")
                nc.vector.scalar_tensor_tensor(out=a_sb[0:1, 0:1], in0=ab_row[0:1, 0:1],
                                               scalar=0.0, in1=ab_row[0:1, 1:2],
                                               op0=OP.add, op1=OP.subtract)
                nc.vector.tensor_copy(a_sb[0:1, 1:2], ab_row[0:1, 2:3])
                nc.sync.dma_start(out=ce_in[N:N + 2], in_=a_sb[0:1, :])

                nc.gpsimd.collective_compute(
                    "AllReduce", OP.add, replica_groups=GROUPS,
                    ins=[ce_in[:].opt()], outs=[ce_out[:].opt()])

                # LT = sum_j log S'_j ; loss = (a + LT - b) / (2N)
                nc.sync.dma_start(out=SY[:], in_=ce_out[0:N].rearrange("(p f) -> p f", p=64))
                ab_fin = main.tile([1, 2], f32, name="ab_fin")
                nc.sync.dma_start(out=ab_fin[0:1, :], in_=ce_out[N:N + 2])
                gY = main.tile([64, N // 64], f32, name="gY")
                ltY = main.tile([64, 1], f32, name="ltY")
                nc.scalar.activation(gY[:], SY[:], AF.Ln, bias=zb[0:64, :], accum_out=ltY[:])
                ps_lt = ceab.tile([1, 1], f32, name="ps_lt")
                nc.tensor.matmul(ps_lt[:], ones32[0:64, :], ltY[:], start=True, stop=True)
                lt_row = main.tile([1, 1], f32, name="lt_row")
                nc.scalar.copy(lt_row[:], ps_lt[:])
                nc.vector.scalar_tensor_tensor(out=loss_sb[:], in0=ab_fin[0:1, 0:1],
                                               scalar=0.0, in1=lt_row[:],
                                               op0=OP.add, op1=OP.add)
                nc.vector.scalar_tensor_tensor(out=loss_sb[:], in0=loss_sb[:],
                                               scalar=0.0, in1=ab_fin[0:1, 1:2],
                                               op0=OP.add, op1=OP.subtract)
                nc.scalar.mul(loss_sb[:], loss_sb[:], 1.0 / (2.0 * N))
                nc.sync.dma_start(out=out_d[:], in_=loss_sb[:])
                dbg = main.tile([128, 288], f32, name="dbg") if DBG else None
                if DBG:
                    nc.vector.memset(dbg[:], 0.0)
                    nc.vector.tensor_copy(dbg[:, 0:T], r_tl[:])
                    nc.vector.tensor_copy(dbg[:, 8:8 + T], cp_tl[:])
                    nc.vector.tensor_copy(dbg[0:64, 16:16 + N // 64], cY[:])
                    nc.vector.tensor_copy(dbg[0:64, 144:144 + N // 64], rpY[:])
                    nc.vector.tensor_copy(dbg[:, 272:272 + T], S8[:])
                    nc.vector.tensor_copy(dbg[:, 280:284], ab[:])
                    nc.vector.tensor_copy(dbg[0:1, 284:288], ab_row[:])
                    nc.vector.tensor_copy(dbg[0:1, 270:271], lt_row[:])
                    nc.sync.dma_start(out=dbg_d[:], in_=dbg[:])

            scr_pool.__exit__(None, None, None)

    nc.finalize()
    return nc


def kernel(all_image_features, all_text_features, logit_scale, labels=None,
           **kwargs):
    import ml_dtypes
    from concourse.bass_utils import run_bass_kernel_spmd

    s = float(np.asarray(logit_scale))
    img = np.asarray(all_image_features, np.float32)
    txt = np.asarray(all_text_features, np.float32)

    bf = ml_dtypes.bfloat16
    imgb = img.astype(bf)
    txtb = txt.astype(bf)

    # host-side diag of P0 (from the bf16-cast features, like the device path)
    dg = np.exp(s * np.einsum("nd,nd->n", imgb.astype(np.float32),
                              txtb.astype(np.float32)) - 1.0).astype(np.float32)

    ident64 = np.eye(64, dtype=np.float32)
    ones32 = np.ones([128, 1], np.float32)
    onesb = np.ones([128, 1], bf)

    in_maps = []
    for k in range(CORES):
        sl = slice(k * ROWS, (k + 1) * ROWS)
        imgT_k = np.ascontiguousarray(imgb[sl].T)
        ttT = txtb[sl].T                                          # [D, ROWS]
        txtTs_k = np.stack([np.ascontiguousarray(ttT[:, :CH]),
                            np.ascontiguousarray(ttT[:, CH:])])   # [2, D, CH]
        diag_k = np.ascontiguousarray(dg[sl].reshape(T, 128).T)   # [p, t]
        premask_k = np.zeros([128, N // 128], np.float32)
        premask_k[:, k * T:(k + 1) * T] = 1.0
        in_maps.append({
            "imgT": imgT_k, "txtTs": txtTs_k, "diagp0": diag_k,
            "premask": premask_k, "ident64": ident64,
            "ones32": ones32, "onesb": onesb,
        })

    global LAST_NC, LAST_IN_MAPS, LAST_RESULTS, _NC_KEY
    key = (s, int(kwargs.get('_iters', MAX_ITERS)), bool(kwargs.get('_ce', True)),
           bool(kwargs.get('_noar', False)))
    if globals().get('_NC_KEY') == key and globals().get('LAST_NC') is not None:
        nc = LAST_NC
    else:
        nc = _build(s, iters=key[1], ce=key[2], NO_AR=key[3])
        _NC_KEY = key
    LAST_NC, LAST_IN_MAPS = nc, in_maps
    res = run_bass_kernel_spmd(nc, in_maps, list(range(CORES)))
    LAST_RESULTS = res
    return np.float32(res.results[0]["out"][0, 0])


if __name__ == "__main__":
    import reference
    inputs = reference.setup_inputs()
    out = kernel(**{k: np.asarray(v) for k, v in inputs.items()})
    print("kernel loss:", out)

